# revision 4
# baseline (speedup 1.0000x reference)
"""MultiHeadDebiasedAttention TRN2 Bass kernel.

Sharding: 8 cores = 4 batches x 2 head-groups (8 heads each).
Per core, scores are computed transposed (S^T[k,q] per head, two heads
row-tiled in the PE array) so the context matmul contracts over k with
full 128 partitions. V is augmented with a ones column so every context
matmul also produces the softmax row-sum for free. Unnormalized
exp(scores)^T tiles stream to HBM; the host transposes + normalizes
them into attention_probs. Softmax max-subtraction is skipped: scores
are O(1) for these inputs and exp stays in fp32 range.
"""
import math
import os

import numpy as np

import concourse.bass as bass
import concourse.tile as tile
import concourse.mybir as mybir
from concourse import bacc
from concourse.bass import ds
from concourse.bass_utils import run_bass_kernel_spmd

B, S, H, NH, HD = 4, 2048, 1024, 16, 64
NCORES = 8
HEADS_PER_CORE = NH // 2          # 8
PAIRS = HEADS_PER_CORE // 2       # 4
P = 128
QB = 512                          # q block
NQB = S // QB                     # 4
NKT = S // P                      # 16 k tiles
NHT = H // P                      # 8 hidden tiles
HHT = NHT // 2                    # 4 hidden tiles per half pass
HD1 = HD + 1                      # V column width incl ones column

f32 = mybir.dt.float32
f32r = mybir.dt.float32r
EXP = mybir.ActivationFunctionType.Exp

_CACHE = {}


def build(use_bias_qk: bool, use_bias_v: bool, use_mask: bool, loop_n: int = 1):
    nc = bacc.Bacc("TRN2", target_bir_lowering=False, debug=False,
                   num_devices=NCORES)

    hsT_in = nc.dram_tensor("hsT", [H, S], f32, kind="ExternalInput").ap()
    wqT_in = nc.dram_tensor("wqT", [H, 512], f32, kind="ExternalInput").ap()
    wkT_in = nc.dram_tensor("wkT", [H, 512], f32, kind="ExternalInput").ap()
    wvT_in = nc.dram_tensor("wvT", [H, 512], f32, kind="ExternalInput").ap()
    woT_in = nc.dram_tensor("woT", [512, H], f32, kind="ExternalInput").ap()
    negc_in = nc.dram_tensor("negc", [P, 8], f32, kind="ExternalInput").ap()
    bq_in = nc.dram_tensor("bq", [512], f32, kind="ExternalInput").ap()
    bk_in = nc.dram_tensor("bk", [512], f32, kind="ExternalInput").ap()
    bv_in = nc.dram_tensor("bv", [1, 512], f32, kind="ExternalInput").ap()
    mask_in = nc.dram_tensor("maskf", [P, NKT], f32, kind="ExternalInput").ap()

    e_out = nc.dram_tensor("e_out", [HEADS_PER_CORE, NQB, NKT, P, QB], f32,
                           kind="ExternalOutput").ap()
    inv_out = nc.dram_tensor("inv_out", [HEADS_PER_CORE, NQB, QB], f32,
                             kind="ExternalOutput").ap()
    y_out = nc.dram_tensor("y_out", [S, H], f32, kind="ExternalOutput").ap()

    hsT_src = hsT_in.rearrange("(t p) s -> p t s", p=P).bitcast(f32r)

    with tile.TileContext(nc) as tc:
        import contextlib
        loop_cm = tc.For_i(0, loop_n, 1) if loop_n > 1 else contextlib.nullcontext()
        with loop_cm, \
             tc.tile_pool(name="stat", bufs=1) as stat, \
             tc.tile_pool(name="dramp", bufs=2, space="DRAM") as dramp:
            # ---- persistent tensors (phases A..C) ----
            negc_sb = stat.tile([P, 8], f32, tag="negc")
            nc.sync.dma_start(negc_sb[:], negc_in)
            ones_sb = stat.tile([P, 1], f32, tag="ones")
            nc.vector.memset(ones_sb[:], 1.0)
            if use_mask:
                mask_sb = stat.tile([P, NKT], f32, tag="mask")
                nc.sync.dma_start(mask_sb[:], mask_in)

            v_sb = stat.tile([P, NKT, 8 * HD1], f32r, tag="v")
            v4d = v_sb.rearrange("p t (h d) -> p t h d", d=HD1)
            nc.vector.tensor_copy(
                v4d[:, :, :, HD:HD1],
                ones_sb[:, :, None, None].to_broadcast((P, NKT, 8, 1)))
            qT_sb = stat.tile([P, PAIRS, S], f32r, tag="qT")
            kT_sb = stat.tile([P, PAIRS, S], f32r, tag="kT")

            # ================= phase A: projections =================
            with tc.tile_pool(name="projA", bufs=1) as projA, \
                 tc.tile_pool(name="pp", bufs=2, space="PSUM") as pp:
                wvT_sb = projA.tile([P, NHT, 512], f32r, tag="wvT")
                nc.sync.dma_start(
                    wvT_sb[:],
                    wvT_in.rearrange("(t p) j -> p t j", p=P).bitcast(f32r))
                wqT_sb = projA.tile([P, NHT, 512], f32r, tag="wqT")
                nc.sync.dma_start(
                    wqT_sb[:],
                    wqT_in.rearrange("(t p) j -> p t j", p=P).bitcast(f32r))
                wkT_sb = projA.tile([P, NHT, 512], f32r, tag="wkT")
                nc.sync.dma_start(
                    wkT_sb[:],
                    wkT_in.rearrange("(t p) j -> p t j", p=P).bitcast(f32r))
                if use_bias_qk:
                    bq_sb = stat.tile([P, PAIRS], f32, tag="bq")
                    nc.sync.dma_start(bq_sb[:],
                                      bq_in.rearrange("(g p) -> p g", p=P))
                    bk_sb = stat.tile([P, PAIRS], f32, tag="bk")
                    nc.sync.dma_start(bk_sb[:],
                                      bk_in.rearrange("(g p) -> p g", p=P))
                if use_bias_v:
                    bv_sb = stat.tile([P, 512], f32, tag="bv")
                    nc.sync.dma_start(bv_sb[:], bv_in.to_broadcast((P, 512)))

                # hidden dim streamed in two halves; second half accumulates
                for half in range(2):
                    hs_half = projA.tile([P, HHT, S], f32r, tag="hsT",
                                         name=f"hsT{half}")
                    nc.sync.dma_start(
                        hs_half[:], hsT_src[:, ds(half * HHT, HHT), :])
                    # V (natural [s, j] layout)
                    for st in range(NKT):
                        v_ps = pp.tile([P, 512], f32, tag="pp",
                                       name=f"vps{half}{st}")
                        for kt in range(HHT):
                            nc.tensor.matmul(
                                v_ps[:],
                                hs_half[:, kt, ds(st * P, P)],
                                wvT_sb[:, half * HHT + kt, :],
                                start=(kt == 0), stop=(kt == HHT - 1),
                            )
                        dst = v4d[:, st, :, 0:HD]
                        src = v_ps.rearrange("p (h d) -> p h d", d=HD)
                        if half == 0:
                            if use_bias_v:
                                nc.vector.tensor_add(
                                    dst, src,
                                    bv_sb.rearrange("p (h d) -> p h d", d=HD))
                            else:
                                nc.vector.tensor_copy(dst, src)
                        else:
                            nc.vector.tensor_add(dst, dst.bitcast(f32), src)
                    # Q^T / K^T ([j, s] layout)
                    for wsb, osb, which in ((wqT_sb, qT_sb, "q"),
                                            (wkT_sb, kT_sb, "k")):
                        for g in range(PAIRS):
                            for qb in range(NQB):
                                q_ps = pp.tile([P, QB], f32, tag="pp",
                                               name=f"ps{which}{half}{g}{qb}")
                                for kt in range(HHT):
                                    nc.tensor.matmul(
                                        q_ps[:],
                                        wsb[:, half * HHT + kt, ds(g * P, P)],
                                        hs_half[:, kt, ds(qb * QB, QB)],
                                        start=(kt == 0), stop=(kt == HHT - 1),
                                    )
                                dst = osb[:, g, ds(qb * QB, QB)]
                                if half == 0:
                                    if use_bias_qk:
                                        bias = (bq_sb if which == "q"
                                                else bk_sb)[:, g:g + 1]
                                        nc.vector.tensor_scalar_add(
                                            dst, q_ps[:], bias)
                                    else:
                                        nc.vector.tensor_copy(dst, q_ps[:])
                                else:
                                    nc.vector.tensor_add(
                                        dst, dst.bitcast(f32), q_ps[:])

            # ================= phase B: attention =================
            with tc.tile_pool(name="ctp", bufs=1) as ctpool:
                ct_sb = ctpool.tile([P, PAIRS, S], f32r, tag="ct")
                with tc.tile_pool(name="ebufp", bufs=2) as ebuf, \
                     tc.tile_pool(name="small", bufs=2) as small, \
                     tc.tile_pool(name="spsp", bufs=2, space="PSUM") as sps, \
                     tc.tile_pool(name="cpsp", bufs=2, space="PSUM") as cps:
                    for g in range(PAIRS):
                        for qb in range(NQB):
                            c_ps = [cps.tile([HD1, QB], f32, tag="cps",
                                             name=f"cps{g}{qb}{h}")
                                    for h in range(2)]
                            for eh in range(2):  # halves of the k range
                                e_t = [ebuf.tile([P, NKT // 2, QB], f32r,
                                                 tag="ebuf",
                                                 name=f"eb{g}{qb}{eh}{h}")
                                       for h in range(2)]
                                for t2 in range(NKT // 4):
                                    s_ps = [sps.tile([P, 2, QB], f32,
                                                     tag="sps",
                                                     name=f"sp{g}{qb}{eh}{t2}{h}")
                                            for h in range(2)]
                                    for dt in range(2):
                                        t = eh * (NKT // 2) + 2 * t2 + dt
                                        for h in range(2):
                                            nc.tensor.matmul(
                                                s_ps[h][:, dt, :],
                                                kT_sb[ds(64 * h, 64), g,
                                                      ds(t * P, P)],
                                                qT_sb[ds(64 * h, 64), g,
                                                      ds(qb * QB, QB)],
                                                start=True, stop=True,
                                            )
                                    for h in range(2):
                                        hh = 2 * g + h
                                        nc.scalar.activation(
                                            e_t[h][:, ds(2 * t2, 2), :],
                                            s_ps[h][:], EXP,
                                            bias=negc_sb[:, hh:hh + 1],
                                            scale=1.0 / math.sqrt(HD),
                                        )
                                        if use_mask:
                                            for dt in range(2):
                                                t = (eh * (NKT // 2)
                                                     + 2 * t2 + dt)
                                                nc.vector.tensor_scalar_mul(
                                                    e_t[h][:, 2 * t2 + dt, :],
                                                    e_t[h][:, 2 * t2 + dt, :],
                                                    mask_sb[:, t:t + 1])
                                    for dt in range(2):
                                        t = eh * (NKT // 2) + 2 * t2 + dt
                                        for h in range(2):
                                            nc.tensor.matmul(
                                                c_ps[h][:],
                                                v_sb[:, t,
                                                     ds((2 * g + h) * HD1, HD1)],
                                                e_t[h][:, 2 * t2 + dt, :],
                                                start=(t == 0),
                                                stop=(t == NKT - 1),
                                            )
                                for h in range(2):
                                    hh = 2 * g + h
                                    nc.sync.dma_start(
                                        e_out[hh, qb,
                                              ds(eh * (NKT // 2), NKT // 2)]
                                        .rearrange("t p q -> p t q")
                                        .bitcast(f32r),
                                        e_t[h][:])
                            # row-sum reciprocals; stream out; normalize ctx
                            inv_sb = [small.tile([1, QB], f32, tag=f"inv{h}",
                                                 name=f"inv{g}{qb}{h}")
                                      for h in range(2)]
                            inv_dt = dramp.tile([2, QB], f32, tag="invd",
                                                name=f"invd{g}{qb}")
                            for h in range(2):
                                hh = 2 * g + h
                                nc.vector.reciprocal(inv_sb[h][:],
                                                     c_ps[h][HD:HD1, :])
                                nc.sync.dma_start(inv_out[hh, qb][None, :],
                                                  inv_sb[h][:])
                                nc.sync.dma_start(inv_dt[h:h + 1, :],
                                                  inv_sb[h][:])
                            ibc = small.tile([P, QB], f32, tag="ibc",
                                             name=f"ibc{g}{qb}")
                            nc.sync.dma_start(
                                ibc[0:HD, :],
                                inv_dt[0:1, :].to_broadcast((HD, QB)))
                            nc.sync.dma_start(
                                ibc[HD:P, :],
                                inv_dt[1:2, :].to_broadcast((HD, QB)))
                            ctp = small.tile([P, QB], f32, tag="ctp",
                                             name=f"ctp{g}{qb}")
                            c1b = small.tile([HD, QB], f32, tag="c1b",
                                             name=f"c1b{g}{qb}")
                            nc.vector.tensor_copy(ctp[0:HD, :],
                                                  c_ps[0][0:HD, :])
                            nc.vector.tensor_copy(c1b[:], c_ps[1][0:HD, :])
                            nc.sync.dma_start(ctp[HD:P, :], c1b[:])
                            nc.vector.tensor_mul(
                                ct_sb[:, g, ds(qb * QB, QB)], ctp[:], ibc[:])

                # ================= phase C: output projection =============
                with tc.tile_pool(name="phC", bufs=1) as phC, \
                     tc.tile_pool(name="ysbp", bufs=2) as ysbp, \
                     tc.tile_pool(name="ypp", bufs=2, space="PSUM") as ypp:
                    woT_sb = phC.tile([P, PAIRS, H], f32r, tag="woT")
                    nc.sync.dma_start(
                        woT_sb[:],
                        woT_in.rearrange("(g p) o -> p g o", p=P).bitcast(f32r))
                    for qt in range(NKT):
                        for ob in range(2):
                            y_ps = ypp.tile([P, QB], f32, tag="yp",
                                            name=f"yps{qt}{ob}")
                            for g in range(PAIRS):
                                nc.tensor.matmul(
                                    y_ps[:],
                                    ct_sb[:, g, ds(qt * P, P)],
                                    woT_sb[:, g, ds(ob * QB, QB)],
                                    start=(g == 0), stop=(g == PAIRS - 1),
                                )
                            y_sb = ysbp.tile([P, QB], f32, tag="ysb",
                                             name=f"ysb{qt}{ob}")
                            nc.vector.tensor_copy(y_sb[:], y_ps[:])
                            nc.sync.dma_start(
                                y_out[ds(qt * P, P), ds(ob * QB, QB)],
                                y_sb[:])

    nc.compile()
    return nc


def make_in_maps(hidden_states, attention_mask, Wq, bq, Wk, bk, Wv, bv, Wo,
                 bias_correction, use_mask):
    WqT, WkT, WvT = Wq.T.copy(), Wk.T.copy(), Wv.T.copy()
    WoT = Wo.T.copy()
    in_maps = []
    for c in range(NCORES):
        b, g2 = c % B, c // B
        hs = slice(g2 * 8 * HD, (g2 * 8 + 8) * HD)
        negc = np.broadcast_to(-bias_correction[g2 * 8:(g2 + 1) * 8][None, :],
                               (P, 8)).copy()
        if use_mask:
            mf = attention_mask[b].astype(np.float32).reshape(NKT, P).T.copy()
        else:
            mf = np.ones((P, NKT), np.float32)
        in_maps.append({
            "hsT": np.ascontiguousarray(hidden_states[b].T),
            "wqT": np.ascontiguousarray(WqT[:, hs]),
            "wkT": np.ascontiguousarray(WkT[:, hs]),
            "wvT": np.ascontiguousarray(WvT[:, hs]),
            "woT": np.ascontiguousarray(WoT[hs, :]),
            "negc": negc,
            "bq": np.ascontiguousarray(bq[hs]),
            "bk": np.ascontiguousarray(bk[hs]),
            "bv": np.ascontiguousarray(bv[hs])[None, :],
            "maskf": mf,
        })
    return in_maps


def kernel(hidden_states, attention_mask, Wq, bq, Wk, bk, Wv, bv, Wo, bo,
           bias_correction):
    hidden_states = np.asarray(hidden_states, dtype=np.float32)
    attention_mask = np.asarray(attention_mask)
    Wq, bq = np.asarray(Wq, np.float32), np.asarray(bq, np.float32)
    Wk, bk = np.asarray(Wk, np.float32), np.asarray(bk, np.float32)
    Wv, bv = np.asarray(Wv, np.float32), np.asarray(bv, np.float32)
    Wo, bo = np.asarray(Wo, np.float32), np.asarray(bo, np.float32)
    bias_correction = np.asarray(bias_correction, np.float32)

    use_bias_qk = bool(np.any(bq) or np.any(bk))
    use_bias_v = bool(np.any(bv))
    use_mask = not bool(np.all(attention_mask == 1))

    key = (use_bias_qk, use_bias_v, use_mask)
    if key not in _CACHE:
        _CACHE[key] = build(*key)
    nc = _CACHE[key]

    in_maps = make_in_maps(hidden_states, attention_mask, Wq, bq, Wk, bk,
                           Wv, bv, Wo, bias_correction, use_mask)

    WoT = Wo.T.copy()
    trace = os.environ.get("ATTN_TRACE", "0") == "1"
    res = run_bass_kernel_spmd(nc, in_maps, list(range(NCORES)), trace=trace)
    kernel.last_result = res

    output = np.empty((B, S, H), np.float32)
    probs = np.empty((B, NH, S, S), np.float32)
    for c in range(NCORES):
        b, g2 = c % B, c // B
        r = res.results[c]
        if g2 == 0:
            output[b] = r["y_out"]
        else:
            output[b] += r["y_out"]
        e = np.asarray(r["e_out"])      # [8, NQB, NKT, P, QB]
        inv = np.asarray(r["inv_out"])  # [8, NQB, QB]
        # probs[b, head, q, k] = e[h, qb, t, p, j] * inv[h, qb, j]
        pr = e.transpose(0, 1, 4, 2, 3).reshape(8, S, S)
        pr = pr * inv.reshape(8, S, 1)
        probs[b, g2 * 8:(g2 + 1) * 8] = pr
    output += bo[None, None, :]
    return (output, probs)


# revision 5
# speedup vs baseline: 2.1884x; 2.1884x over previous
"""MultiHeadDebiasedAttention TRN2 Bass kernel.

Sharding: 8 cores = 4 batches x 2 head-groups (8 heads each).
Per core, scores are computed transposed (S^T[k,q] per head, two heads
row-tiled in the PE array) so the context matmul contracts over k with
full 128 partitions. V is augmented with a ones column so every context
matmul also produces the softmax row-sum for free. Unnormalized
exp(scores)^T tiles stream to HBM; the host transposes + normalizes
them into attention_probs. Softmax max-subtraction is skipped: scores
are O(1) for these inputs and exp stays in fp32 range.
"""
import math
import os

import numpy as np

import concourse.bass as bass
import concourse.tile as tile
import concourse.mybir as mybir
from concourse import bacc
from concourse.bass import ds
from concourse.bass_utils import run_bass_kernel_spmd

B, S, H, NH, HD = 4, 2048, 1024, 16, 64
NCORES = 8
HEADS_PER_CORE = NH // 2          # 8
PAIRS = HEADS_PER_CORE // 2       # 4
P = 128
QB = 512                          # q block
NQB = S // QB                     # 4
NKT = S // P                      # 16 k tiles
NHT = H // P                      # 8 hidden tiles
HHT = NHT // 2                    # 4 hidden tiles per half pass
HD1 = HD + 1                      # V column width incl ones column

f32 = mybir.dt.float32
f32r = mybir.dt.float32r
EXP = mybir.ActivationFunctionType.Exp

_CACHE = {}


def build(use_bias_qk: bool, use_bias_v: bool, use_mask: bool, loop_n: int = 1):
    nc = bacc.Bacc("TRN2", target_bir_lowering=False, debug=False,
                   num_devices=NCORES)

    hsT_in = nc.dram_tensor("hsT", [H, S], f32, kind="ExternalInput").ap()
    wqT_in = nc.dram_tensor("wqT", [H, 512], f32, kind="ExternalInput").ap()
    wkT_in = nc.dram_tensor("wkT", [H, 512], f32, kind="ExternalInput").ap()
    wvT_in = nc.dram_tensor("wvT", [H, 512], f32, kind="ExternalInput").ap()
    woT_in = nc.dram_tensor("woT", [512, H], f32, kind="ExternalInput").ap()
    negc_in = nc.dram_tensor("negc", [P, 8], f32, kind="ExternalInput").ap()
    bq_in = nc.dram_tensor("bq", [512], f32, kind="ExternalInput").ap()
    bk_in = nc.dram_tensor("bk", [512], f32, kind="ExternalInput").ap()
    bv_in = nc.dram_tensor("bv", [1, 512], f32, kind="ExternalInput").ap()
    mask_in = nc.dram_tensor("maskf", [P, NKT], f32, kind="ExternalInput").ap()

    e_out = nc.dram_tensor("e_out", [HEADS_PER_CORE, NQB, P, NKT, QB], f32,
                           kind="ExternalOutput").ap()
    inv_out = nc.dram_tensor("inv_out", [HEADS_PER_CORE, NQB, QB], f32,
                             kind="ExternalOutput").ap()
    y_out = nc.dram_tensor("y_out", [S, H], f32, kind="ExternalOutput").ap()

    hsT_src = hsT_in.rearrange("(t p) s -> p t s", p=P).bitcast(f32r)

    with tile.TileContext(nc) as tc:
        import contextlib
        loop_cm = tc.For_i(0, loop_n, 1) if loop_n > 1 else contextlib.nullcontext()
        with loop_cm, \
             tc.tile_pool(name="stat", bufs=1) as stat, \
             tc.tile_pool(name="dramp", bufs=2, space="DRAM") as dramp:
            # ---- persistent tensors (phases A..C) ----
            negc_sb = stat.tile([P, 8], f32, tag="negc")
            nc.sync.dma_start(negc_sb[:], negc_in)
            ones_sb = stat.tile([P, 1], f32, tag="ones")
            nc.vector.memset(ones_sb[:], 1.0)
            if use_mask:
                mask_sb = stat.tile([P, NKT], f32, tag="mask")
                nc.sync.dma_start(mask_sb[:], mask_in)

            v_sb = stat.tile([P, NKT, 8 * HD1], f32r, tag="v")
            v4d = v_sb.rearrange("p t (h d) -> p t h d", d=HD1)
            nc.vector.tensor_copy(
                v4d[:, :, :, HD:HD1],
                ones_sb[:, :, None, None].to_broadcast((P, NKT, 8, 1)))
            qT_sb = stat.tile([P, PAIRS, S], f32r, tag="qT")
            kT_sb = stat.tile([P, PAIRS, S], f32r, tag="kT")

            # ================= phase A: projections =================
            with tc.tile_pool(name="projA", bufs=1) as projA, \
                 tc.tile_pool(name="pp", bufs=2, space="PSUM") as pp:
                wvT_sb = projA.tile([P, NHT, 512], f32r, tag="wvT")
                nc.sync.dma_start(
                    wvT_sb[:],
                    wvT_in.rearrange("(t p) j -> p t j", p=P).bitcast(f32r))
                wqT_sb = projA.tile([P, NHT, 512], f32r, tag="wqT")
                nc.sync.dma_start(
                    wqT_sb[:],
                    wqT_in.rearrange("(t p) j -> p t j", p=P).bitcast(f32r))
                wkT_sb = projA.tile([P, NHT, 512], f32r, tag="wkT")
                nc.sync.dma_start(
                    wkT_sb[:],
                    wkT_in.rearrange("(t p) j -> p t j", p=P).bitcast(f32r))
                if use_bias_qk:
                    bq_sb = stat.tile([P, PAIRS], f32, tag="bq")
                    nc.sync.dma_start(bq_sb[:],
                                      bq_in.rearrange("(g p) -> p g", p=P))
                    bk_sb = stat.tile([P, PAIRS], f32, tag="bk")
                    nc.sync.dma_start(bk_sb[:],
                                      bk_in.rearrange("(g p) -> p g", p=P))
                if use_bias_v:
                    bv_sb = stat.tile([P, 512], f32, tag="bv")
                    nc.sync.dma_start(bv_sb[:], bv_in.to_broadcast((P, 512)))

                # hidden dim streamed in two halves; second half accumulates
                for half in range(2):
                    hs_half = projA.tile([P, HHT, S], f32r, tag="hsT",
                                         name=f"hsT{half}")
                    nc.sync.dma_start(
                        hs_half[:], hsT_src[:, ds(half * HHT, HHT), :])
                    # V (natural [s, j] layout)
                    for st in range(NKT):
                        v_ps = pp.tile([P, 512], f32, tag="pp",
                                       name=f"vps{half}{st}")
                        for kt in range(HHT):
                            nc.tensor.matmul(
                                v_ps[:],
                                hs_half[:, kt, ds(st * P, P)],
                                wvT_sb[:, half * HHT + kt, :],
                                start=(kt == 0), stop=(kt == HHT - 1),
                            )
                        dst = v4d[:, st, :, 0:HD]
                        src = v_ps.rearrange("p (h d) -> p h d", d=HD)
                        if half == 0:
                            if use_bias_v:
                                nc.vector.tensor_add(
                                    dst, src,
                                    bv_sb.rearrange("p (h d) -> p h d", d=HD))
                            else:
                                nc.vector.tensor_copy(dst, src)
                        else:
                            nc.vector.tensor_add(dst, dst.bitcast(f32), src)
                    # Q^T / K^T ([j, s] layout)
                    for wsb, osb, which in ((wqT_sb, qT_sb, "q"),
                                            (wkT_sb, kT_sb, "k")):
                        for g in range(PAIRS):
                            for qb in range(NQB):
                                q_ps = pp.tile([P, QB], f32, tag="pp",
                                               name=f"ps{which}{half}{g}{qb}")
                                for kt in range(HHT):
                                    nc.tensor.matmul(
                                        q_ps[:],
                                        wsb[:, half * HHT + kt, ds(g * P, P)],
                                        hs_half[:, kt, ds(qb * QB, QB)],
                                        start=(kt == 0), stop=(kt == HHT - 1),
                                    )
                                dst = osb[:, g, ds(qb * QB, QB)]
                                if half == 0:
                                    if use_bias_qk:
                                        bias = (bq_sb if which == "q"
                                                else bk_sb)[:, g:g + 1]
                                        nc.vector.tensor_scalar_add(
                                            dst, q_ps[:], bias)
                                    else:
                                        nc.vector.tensor_copy(dst, q_ps[:])
                                else:
                                    nc.vector.tensor_add(
                                        dst, dst.bitcast(f32), q_ps[:])

            # ================= phase B: attention =================
            with tc.tile_pool(name="ctp", bufs=1) as ctpool:
                ct_sb = ctpool.tile([P, PAIRS, S], f32r, tag="ct")
                with tc.tile_pool(name="ebufp", bufs=2) as ebuf, \
                     tc.tile_pool(name="small", bufs=2) as small, \
                     tc.tile_pool(name="spsp", bufs=2, space="PSUM") as sps, \
                     tc.tile_pool(name="cpsp", bufs=2, space="PSUM") as cps:
                    for g in range(PAIRS):
                        for qb in range(NQB):
                            c_ps = [cps.tile([HD1, QB], f32, tag="cps",
                                             name=f"cps{g}{qb}{h}")
                                    for h in range(2)]
                            for eh in range(2):  # halves of the k range
                                e_t = [ebuf.tile([P, NKT // 2, QB], f32r,
                                                 tag="ebuf",
                                                 name=f"eb{g}{qb}{eh}{h}")
                                       for h in range(2)]
                                for t2 in range(NKT // 4):
                                    s_ps = [sps.tile([P, 2, QB], f32,
                                                     tag="sps",
                                                     name=f"sp{g}{qb}{eh}{t2}{h}")
                                            for h in range(2)]
                                    for dt in range(2):
                                        t = eh * (NKT // 2) + 2 * t2 + dt
                                        for h in range(2):
                                            nc.tensor.matmul(
                                                s_ps[h][:, dt, :],
                                                kT_sb[ds(64 * h, 64), g,
                                                      ds(t * P, P)],
                                                qT_sb[ds(64 * h, 64), g,
                                                      ds(qb * QB, QB)],
                                                start=True, stop=True,
                                            )
                                    for h in range(2):
                                        hh = 2 * g + h
                                        nc.scalar.activation(
                                            e_t[h][:, ds(2 * t2, 2), :],
                                            s_ps[h][:], EXP,
                                            bias=negc_sb[:, hh:hh + 1],
                                            scale=1.0 / math.sqrt(HD),
                                        )
                                        if use_mask:
                                            for dt in range(2):
                                                t = (eh * (NKT // 2)
                                                     + 2 * t2 + dt)
                                                nc.vector.tensor_scalar_mul(
                                                    e_t[h][:, 2 * t2 + dt, :],
                                                    e_t[h][:, 2 * t2 + dt, :],
                                                    mask_sb[:, t:t + 1])
                                    for dt in range(2):
                                        t = eh * (NKT // 2) + 2 * t2 + dt
                                        for h in range(2):
                                            nc.tensor.matmul(
                                                c_ps[h][:],
                                                v_sb[:, t,
                                                     ds((2 * g + h) * HD1, HD1)],
                                                e_t[h][:, 2 * t2 + dt, :],
                                                start=(t == 0),
                                                stop=(t == NKT - 1),
                                            )
                                for h in range(2):
                                    hh = 2 * g + h
                                    nc.gpsimd.dma_start(
                                        e_out[hh, qb, :,
                                              ds(eh * (NKT // 2), NKT // 2), :]
                                        .bitcast(f32r),
                                        e_t[h][:])
                            # row-sum reciprocals; stream out; normalize ctx
                            inv_sb = [small.tile([1, QB], f32, tag=f"inv{h}",
                                                 name=f"inv{g}{qb}{h}")
                                      for h in range(2)]
                            inv_dt = dramp.tile([2, QB], f32, tag="invd",
                                                name=f"invd{g}{qb}")
                            for h in range(2):
                                hh = 2 * g + h
                                nc.vector.reciprocal(inv_sb[h][:],
                                                     c_ps[h][HD:HD1, :])
                                nc.sync.dma_start(inv_out[hh, qb][None, :],
                                                  inv_sb[h][:])
                                nc.sync.dma_start(inv_dt[h:h + 1, :],
                                                  inv_sb[h][:])
                            ibc = small.tile([P, QB], f32, tag="ibc",
                                             name=f"ibc{g}{qb}")
                            nc.sync.dma_start(
                                ibc[0:HD, :],
                                inv_dt[0:1, :].to_broadcast((HD, QB)))
                            nc.sync.dma_start(
                                ibc[HD:P, :],
                                inv_dt[1:2, :].to_broadcast((HD, QB)))
                            ctp = small.tile([P, QB], f32, tag="ctp",
                                             name=f"ctp{g}{qb}")
                            c1b = small.tile([HD, QB], f32, tag="c1b",
                                             name=f"c1b{g}{qb}")
                            nc.vector.tensor_copy(ctp[0:HD, :],
                                                  c_ps[0][0:HD, :])
                            nc.vector.tensor_copy(c1b[:], c_ps[1][0:HD, :])
                            nc.sync.dma_start(ctp[HD:P, :], c1b[:])
                            nc.vector.tensor_mul(
                                ct_sb[:, g, ds(qb * QB, QB)], ctp[:], ibc[:])

                # ================= phase C: output projection =============
                with tc.tile_pool(name="phC", bufs=1) as phC, \
                     tc.tile_pool(name="ysbp", bufs=2) as ysbp, \
                     tc.tile_pool(name="ypp", bufs=2, space="PSUM") as ypp:
                    woT_sb = phC.tile([P, PAIRS, H], f32r, tag="woT")
                    nc.sync.dma_start(
                        woT_sb[:],
                        woT_in.rearrange("(g p) o -> p g o", p=P).bitcast(f32r))
                    for qt in range(NKT):
                        for ob in range(2):
                            y_ps = ypp.tile([P, QB], f32, tag="yp",
                                            name=f"yps{qt}{ob}")
                            for g in range(PAIRS):
                                nc.tensor.matmul(
                                    y_ps[:],
                                    ct_sb[:, g, ds(qt * P, P)],
                                    woT_sb[:, g, ds(ob * QB, QB)],
                                    start=(g == 0), stop=(g == PAIRS - 1),
                                )
                            y_sb = ysbp.tile([P, QB], f32, tag="ysb",
                                             name=f"ysb{qt}{ob}")
                            nc.vector.tensor_copy(y_sb[:], y_ps[:])
                            nc.sync.dma_start(
                                y_out[ds(qt * P, P), ds(ob * QB, QB)],
                                y_sb[:])

    nc.compile()
    return nc


def make_in_maps(hidden_states, attention_mask, Wq, bq, Wk, bk, Wv, bv, Wo,
                 bias_correction, use_mask):
    WqT, WkT, WvT = Wq.T.copy(), Wk.T.copy(), Wv.T.copy()
    WoT = Wo.T.copy()
    in_maps = []
    for c in range(NCORES):
        b, g2 = c % B, c // B
        hs = slice(g2 * 8 * HD, (g2 * 8 + 8) * HD)
        negc = np.broadcast_to(-bias_correction[g2 * 8:(g2 + 1) * 8][None, :],
                               (P, 8)).copy()
        if use_mask:
            mf = attention_mask[b].astype(np.float32).reshape(NKT, P).T.copy()
        else:
            mf = np.ones((P, NKT), np.float32)
        in_maps.append({
            "hsT": np.ascontiguousarray(hidden_states[b].T),
            "wqT": np.ascontiguousarray(WqT[:, hs]),
            "wkT": np.ascontiguousarray(WkT[:, hs]),
            "wvT": np.ascontiguousarray(WvT[:, hs]),
            "woT": np.ascontiguousarray(WoT[hs, :]),
            "negc": negc,
            "bq": np.ascontiguousarray(bq[hs]),
            "bk": np.ascontiguousarray(bk[hs]),
            "bv": np.ascontiguousarray(bv[hs])[None, :],
            "maskf": mf,
        })
    return in_maps


def kernel(hidden_states, attention_mask, Wq, bq, Wk, bk, Wv, bv, Wo, bo,
           bias_correction):
    hidden_states = np.asarray(hidden_states, dtype=np.float32)
    attention_mask = np.asarray(attention_mask)
    Wq, bq = np.asarray(Wq, np.float32), np.asarray(bq, np.float32)
    Wk, bk = np.asarray(Wk, np.float32), np.asarray(bk, np.float32)
    Wv, bv = np.asarray(Wv, np.float32), np.asarray(bv, np.float32)
    Wo, bo = np.asarray(Wo, np.float32), np.asarray(bo, np.float32)
    bias_correction = np.asarray(bias_correction, np.float32)

    use_bias_qk = bool(np.any(bq) or np.any(bk))
    use_bias_v = bool(np.any(bv))
    use_mask = not bool(np.all(attention_mask == 1))

    key = (use_bias_qk, use_bias_v, use_mask)
    if key not in _CACHE:
        _CACHE[key] = build(*key)
    nc = _CACHE[key]

    in_maps = make_in_maps(hidden_states, attention_mask, Wq, bq, Wk, bk,
                           Wv, bv, Wo, bias_correction, use_mask)

    WoT = Wo.T.copy()
    trace = os.environ.get("ATTN_TRACE", "0") == "1"
    res = run_bass_kernel_spmd(nc, in_maps, list(range(NCORES)), trace=trace)
    kernel.last_result = res

    output = np.empty((B, S, H), np.float32)
    probs = np.empty((B, NH, S, S), np.float32)
    for c in range(NCORES):
        b, g2 = c % B, c // B
        r = res.results[c]
        if g2 == 0:
            output[b] = r["y_out"]
        else:
            output[b] += r["y_out"]
        e = np.asarray(r["e_out"])      # [8, NQB, P, NKT, QB]
        inv = np.asarray(r["inv_out"])  # [8, NQB, QB]
        # probs[b, head, q=qb*QB+j, k=t*P+p] = e[h, qb, p, t, j] * inv[h, qb, j]
        pr = e.transpose(0, 1, 4, 3, 2).reshape(8, S, S)
        pr = pr * inv.reshape(8, S, 1)
        probs[b, g2 * 8:(g2 + 1) * 8] = pr
    output += bo[None, None, :]
    return (output, probs)
